# revision 25
# baseline (speedup 1.0000x reference)
"""Trainium2 Bass kernel for decode-step multi-head attention with RoPE
re-applied to the full KV cache (nn_MultiHeadAttention_50216757624897).

Sharding: 16 heads tensor-parallel across 8 cores (2 heads/core).
QKV weights split column-wise by head, KV cache split on the head dim,
out-proj row-parallel; partials summed on host (the unshard step).

Architecture (v4, transposed-K, batch-paired rot products, 3-pass fold):
 - K cache host-permuted to [b, (rot|pass, h, d'), s] fp16: partitions are
   the 2x32 rotary dims then the 2x32 passthrough dims; free dim carries
   all 4096 positions.
 - score[s,h] = sum_rot k*cos*u + sum_rot k*sin*v + sum_pass k*u. The
   passthrough sum needs NO elementwise work at all (k itself is the
   stationary operand, u folds into the mask); the rotary halves of TWO
   batches pack into one 128-partition tile, so kcr = krot (.) cos and
   ksr = krot (.) sin each cover a batch PAIR in one DVE-2x/Pool op.
   Per 128-position chunk, three accumulating matmuls per batch (kcr/um,
   ksr/vm, kpass/upm masks) write scores [128 pos, 2 heads] to PSUM.
 - u/v/u_pass per-batch mask columns are built from a tiny DRAM bounce of
   the transposed q (rot rows duplicated into both pair slots).
 - The new (current) token's K rotation cancels with Q's: score_new = qh.kh.
 - Softmax runs without max-subtraction (|score/8| < 3).
 - V cache host-permuted to [b, p, (h, c, d)] fp8-e3m4 (position = c*128+p):
   it is consumed only by PE A.V matmuls (fp8 full-rate), halving its HBM
   traffic; A.V contracts over partitions like the score layout.
 - DMA transfers overlap across issuing queues in the cost model; the kv
   stream alternates SP/Act and the cos/sin tables ship as halves on
   SWDGE/Act so both tables land by ~6us.
"""

import os
import sys
from contextlib import ExitStack

import numpy as np
import ml_dtypes

sys.path.insert(0, "/opt/trn_rl_repo")

import concourse.bass as bass
import concourse.bacc as bacc
import concourse.tile as tile
from concourse import mybir
from concourse.bass_types import AP
from concourse.bass_utils import run_bass_kernel_spmd

F32 = mybir.dt.float32
F16 = mybir.dt.float16
F8 = mybir.dt.float8e3
AF = mybir.ActivationFunctionType
AX = mybir.AxisListType
OP = mybir.AluOpType

BS, NH, HD, ROT, CL, D = 8, 16, 64, 32, 4096, 1024
THETA = 10000.0
N_CORES = 8
H_PER_CORE = NH // N_CORES  # 2
HALF = CL // 2

# (pair, col-half, product[0=kc,1=ks]) triples whose multiply runs on Pool
def _parse_pp(s):
    out = set()
    for item in s.split(";"):
        if item:
            a, b, c = item.split(",")
            out.add((int(a), int(b), int(c)))
    return out

PAIR_POOL = _parse_pp(os.environ.get(
    "PAIR_POOL", "0,0,1;0,1,1;1,0,1;1,1,1;2,0,1;2,1,1"))


def _fap(t, off, dims):
    """AP over tile t with the tile's partition dim, extra free-dim spec."""
    b = t[:]
    return AP(tensor=b.tensor, offset=b.offset + off, ap=[list(b.ap[0])] + dims)


def _rotap(t, off):
    """[8, 2h, 16pairs] strided view of a [8,128] tile selecting pair elem
    `off` (0=even, 1=odd) of the rotary dims."""
    return _fap(t, off, [[64, 2], [2, 16]])


def build_program():
    nc = bacc.Bacc("TRN2", target_bir_lowering=False, debug=False)

    k_c = nc.dram_tensor("k_c", [BS, 128, CL], F16, kind="ExternalInput")
    v_c = nc.dram_tensor("v_c", [BS, 128, CL], F8, kind="ExternalInput")
    q_t = nc.dram_tensor("q_t", [D, BS], F16, kind="ExternalInput")
    wqkv_t = nc.dram_tensor("wqkv_t", [D, 384], F16, kind="ExternalInput")
    bqkv = nc.dram_tensor("bqkv", [1, 384], F16, kind="ExternalInput")
    wo_t = nc.dram_tensor("wo_t", [128, D], F16, kind="ExternalInput")
    cos_t = nc.dram_tensor("cos_t", [128, CL], F16, kind="ExternalInput")
    sin_t = nc.dram_tensor("sin_t", [128, CL], F16, kind="ExternalInput")
    cq_t = nc.dram_tensor("cq_t", [BS, 128], F32, kind="ExternalInput")
    sq_t = nc.dram_tensor("sq_t", [BS, 128], F32, kind="ExternalInput")
    id8 = nc.dram_tensor("id8", [8, 8], F32, kind="ExternalInput")
    id8f = nc.dram_tensor("id8f", [8, 8], F16, kind="ExternalInput")
    hsel0 = nc.dram_tensor("hsel0", [128, 2], F16, kind="ExternalInput")
    hsel1 = nc.dram_tensor("hsel1", [128, 2], F16, kind="ExternalInput")
    out_p = nc.dram_tensor("out_p", [BS, D], F32, kind="ExternalOutput")

    with tile.TileContext(nc) as tc:
        with ExitStack() as ctx:
            _body(nc, tc, ctx, locals())
    nc.finalize()
    return nc


def _body(nc, tc, ctx, t):
    k_c, v_c = t["k_c"], t["v_c"]
    out_p = t["out_p"]

    const = ctx.enter_context(tc.tile_pool(name="const", bufs=1))
    small = ctx.enter_context(tc.tile_pool(name="small", bufs=1))

    # ---- constants. q/wqkv gate the q-chain (SP); table halves spread over
    # SWDGE + Act so both tables land by ~6us; kv stream follows.
    sb_qt = const.tile([128, 8, 8], F16, tag="qt")
    nc.sync.dma_start(sb_qt[:], t["q_t"].rearrange("(c p) b -> p c b", p=128))
    sb_wqkv = const.tile([128, 8, 384], F16, tag="wqkv")
    nc.sync.dma_start(sb_wqkv[:], t["wqkv_t"].rearrange("(c p) n -> p c n", p=128))

    sb_cos = const.tile([128, CL], F16, tag="cos")
    sb_sin = const.tile([128, CL], F16, tag="sin")
    nc.gpsimd.dma_start(sb_cos[:, 0:HALF], t["cos_t"][:, 0:HALF])
    nc.gpsimd.dma_start(sb_sin[:, 0:HALF], t["sin_t"][:, 0:HALF])

    # batch-0 K/V jump the Act queue so kc_0 can start when the tables land
    kt0 = None  # placeholder; the loop below reuses tiles allocated here
    sb_bqkv = const.tile([1, 384], F16, tag="bqkv")
    nc.scalar.dma_start(sb_bqkv[:], t["bqkv"][:, :])
    sb_cq = const.tile([BS, 128], F32, tag="cq")
    nc.scalar.dma_start(sb_cq[:], t["cq_t"][:, :])
    sb_sq = const.tile([BS, 128], F32, tag="sq")
    nc.scalar.dma_start(sb_sq[:], t["sq_t"][:, :])
    sb_id8 = const.tile([8, 8], F32, tag="id8")
    nc.scalar.dma_start(sb_id8[:], t["id8"][:, :])
    sb_id8f = const.tile([8, 8], F16, tag="id8f")
    nc.scalar.dma_start(sb_id8f[:], t["id8f"][:, :])
    sb_hsel0 = const.tile([128, 2], F16, tag="hsel0")
    nc.scalar.dma_start(sb_hsel0[:], t["hsel0"][:, :])
    sb_hsel1 = const.tile([128, 2], F16, tag="hsel1")
    nc.scalar.dma_start(sb_hsel1[:], t["hsel1"][:, :])
    sb_wo0 = const.tile([64, 1024], F16, tag="wo0")
    nc.scalar.dma_start(sb_wo0[:], t["wo_t"][0:64, :])
    sb_wo1 = const.tile([64, 1024], F16, tag="wo1")
    nc.scalar.dma_start(sb_wo1[:], t["wo_t"][64:128, :])

    ones_p = const.tile([128, 1], F32, tag="ones_p")
    nc.vector.memset(ones_p[:], 1.0)
    ones_r8 = const.tile([1, 8], F16, tag="ones_r8")
    nc.vector.memset(ones_r8[:], 1.0)
    ones_r64 = const.tile([1, 64], F32, tag="ones_r64")
    nc.vector.memset(ones_r64[:], 1.0)

    # ---- q/k/v projection of the new token
    qtr_stack = ExitStack()
    psum_proj = qtr_stack.enter_context(tc.tile_pool(name="psum_proj", bufs=1, space="PSUM"))
    projs = small.tile([8, 384], F32, tag="projs")
    ps_qkv = psum_proj.tile([8, 384], F32, tag="ps_qkv")
    ps_q = ps_qkv[:, 0:128]
    for ci in range(8):
        nc.tensor.matmul(ps_q, lhsT=sb_qt[:, ci, :], rhs=sb_wqkv[:, ci, 0:128],
                         start=(ci == 0), stop=False, skip_group_check=True)
    nc.tensor.matmul(ps_q, lhsT=ones_r8[:], rhs=sb_bqkv[:, 0:128],
                     start=False, stop=True, skip_group_check=True)
    nc.scalar.copy(projs[:, 0:128], ps_q)
    ps_kv = ps_qkv[:, 128:384]
    for ci in range(8):
        nc.tensor.matmul(ps_kv, lhsT=sb_qt[:, ci, :], rhs=sb_wqkv[:, ci, 128:384],
                         start=False, stop=False, skip_group_check=True)
    nc.tensor.matmul(ps_kv, lhsT=ones_r8[:], rhs=sb_bqkv[:, 128:384],
                     start=False, stop=True, skip_group_check=True)
    nc.scalar.copy(projs[:, 128:384], ps_kv)
    qh, kh, vh = projs[:, 0:128], projs[:, 128:256], projs[:, 256:384]

    # ---- RoPE on q (full width: host tables carry [cos|1], [sin|0]); u and
    # v = G(u) side by side in one [8, 256] f16 tile.
    qrv = small.tile([8, 256], F16, tag="qrv")
    qr, vG = qrv[:, 0:128], qrv[:, 128:256]
    Hh = small.tile([8, 128], F32, tag="Hh")
    nc.vector.memset(Hh[:], 0.0)
    nc.vector.tensor_scalar_mul(_rotap(Hh, 0), _fap(ps_qkv, 1, [[64, 2], [2, 16]]), -1.0)
    nc.vector.tensor_copy(_rotap(Hh, 1), _fap(ps_qkv, 0, [[64, 2], [2, 16]]))
    t1q = small.tile([8, 128], F32, tag="t1q")
    nc.vector.tensor_mul(t1q[:], ps_q, sb_cq[:])
    t2q = small.tile([8, 128], F32, tag="t2q")
    nc.vector.tensor_mul(t2q[:], Hh[:], sb_sq[:])
    nc.vector.tensor_add(qr, t2q[:], t1q[:])
    # v = G(q_rot): pairs (x0,x1) -> (x1,-x0); zero elsewhere
    nc.vector.memset(vG, 0.0)
    nc.vector.tensor_copy(_fap(qrv, 128, [[64, 2], [2, 16]]),
                          _fap(qrv, 1, [[64, 2], [2, 16]]))
    nc.vector.tensor_scalar_mul(_fap(qrv, 129, [[64, 2], [2, 16]]),
                                _fap(qrv, 0, [[64, 2], [2, 16]]), -1.0)

    # ---- transpose u, v to [128 (rot|pass, h, d'), 8 b]: lhsT views reorder
    # the free dims so rotary rows land first
    psum_tr = qtr_stack.enter_context(tc.tile_pool(name="psum_tr", bufs=1, space="PSUM"))
    qb = qrv[:]
    qr_ro = AP(tensor=qb.tensor, offset=qb.offset,
               ap=[list(qb.ap[0]), [32, 2], [64, 2], [1, 32]])
    vG_ro = AP(tensor=qb.tensor, offset=qb.offset + 128,
               ap=[list(qb.ap[0]), [32, 2], [64, 2], [1, 32]])
    qro = small.tile([8, 256], F16, tag="qro")
    nc.vector.tensor_copy(qro[:, 0:128], qr_ro)
    nc.vector.tensor_copy(qro[:, 128:256], vG_ro)
    uv_ps = psum_tr.tile([128, 16], F16, tag="uv_ps")
    nc.tensor.matmul(uv_ps[:, 0:8], lhsT=qro[:, 0:128], rhs=sb_id8f[:],
                     is_transpose=True, start=True, stop=False,
                     skip_group_check=True)
    nc.tensor.matmul(uv_ps[:, 8:16], lhsT=qro[:, 128:256], rhs=sb_id8f[:],
                     is_transpose=True, start=False, stop=True,
                     skip_group_check=True)
    uv_T = small.tile([128, 16], F32, tag="uv_T")
    nc.scalar.copy(uv_T[:], uv_ps[:])

    # bounce through DRAM to duplicate the rot/pass row groups into both
    # pair slots: urot2/vrot2/upass2 [128, 8] with rows [grp | grp]
    uvd = nc.dram_tensor("uv_scratch", [128, 16], F32, kind="Internal")
    uvb = uv_T[:]
    nc.sync.dma_start(uvd[:, :], uv_T[:])
    dup = small.tile([128, 24], F32, tag="dup")
    dsrc = uvd[:, :]
    # urot2: dram rows 0:64 (u rot) duplicated
    nc.sync.dma_start(dup[:, 0:8], AP(tensor=dsrc.tensor, offset=dsrc.offset,
                                      ap=[[0, 2], [16, 64], [1, 8]]))
    # vrot2: dram rows 0:64 of the v half (offset 8)
    nc.sync.dma_start(dup[:, 8:16], AP(tensor=dsrc.tensor, offset=dsrc.offset + 8,
                                       ap=[[0, 2], [16, 64], [1, 8]]))
    # upass2: dram rows 64:128 of u
    nc.sync.dma_start(dup[:, 16:24], AP(tensor=dsrc.tensor, offset=dsrc.offset + 64 * 16,
                                        ap=[[0, 2], [16, 64], [1, 8]]))

    # per-batch mask columns [128, 2]: um/vm on the rot rows of the batch's
    # pair slot, upm on the pass rows
    um = small.tile([128, 16], F16, tag="um")
    vm = small.tile([128, 16], F16, tag="vm")
    upm = small.tile([128, 16], F16, tag="upm")
    for b in range(8):
        hsel = sb_hsel0 if b % 2 == 0 else sb_hsel1
        nc.vector.tensor_scalar(um[:, 2 * b:2 * b + 2], hsel[:],
                                dup[:, b:b + 1], None, OP.mult)
        nc.vector.tensor_scalar(vm[:, 2 * b:2 * b + 2], hsel[:],
                                dup[:, 8 + b:8 + b + 1], None, OP.mult)
        nc.vector.tensor_scalar(upm[:, 2 * b:2 * b + 2], hsel[:],
                                dup[:, 16 + b:16 + b + 1], None, OP.mult)

    # ---- new-token score: rotations cancel -> qh . kh
    sn = small.tile([8, 128], F32, tag="sn")
    nc.vector.tensor_mul(sn[:], qh, kh)
    scn = small.tile([8, 2], F32, tag="scn")
    nc.vector.reduce_sum(scn[:], _fap(sn, 0, [[64, 2], [1, 64]]), axis=AX.X)
    expn = small.tile([8, 2], F32, tag="expn")
    nc.scalar.activation(expn[:], scn[:], AF.Exp, scale=0.125)
    vhs = small.tile([8, 128], F32, tag="vhs")
    nc.vector.tensor_mul(_fap(vhs, 0, [[64, 2], [1, 64]]),
                         _fap(projs, 256, [[64, 2], [1, 64]]),
                         _fap(expn, 0, [[1, 2], [0, 64]]))

    qtr_stack.close()  # release proj/transpose PSUM banks for the loop pools

    # ---- main per-pair loop
    krpool = ctx.enter_context(tc.tile_pool(name="krpool", bufs=2))
    kppool = ctx.enter_context(tc.tile_pool(name="kppool", bufs=2))
    vpool = ctx.enter_context(tc.tile_pool(name="vpool", bufs=4))
    kcpool = ctx.enter_context(tc.tile_pool(name="kcpool", bufs=2))
    kspool = ctx.enter_context(tc.tile_pool(name="kspool", bufs=2))
    apool = ctx.enter_context(tc.tile_pool(name="apool", bufs=3))
    psum_sc = ctx.enter_context(tc.tile_pool(name="psum_sc", bufs=3, space="PSUM"))
    psum_r = ctx.enter_context(tc.tile_pool(name="psum_r", bufs=1, space="PSUM"))
    psum_wo = ctx.enter_context(tc.tile_pool(name="psum_wo", bufs=2, space="PSUM"))
    psum_main = ctx.enter_context(tc.tile_pool(name="psum_main", bufs=1, space="PSUM"))

    ov_ps = psum_main.tile([64, 16], F32, tag="ov")
    den_ps = psum_main.tile([1, 16], F32, tag="den")
    den_part = small.tile([128, 16], F32, tag="den_part")

    # init PSUM with the new-token contribution (transposes of vh*exp, exp)
    # NOTE: PSUM start=True zeroes the whole 2KB bank row, so only the FIRST
    # write into each psum tile may use start=True.
    for h in range(H_PER_CORE):
        nc.tensor.matmul(ov_ps[:, h * 8:(h + 1) * 8], lhsT=vhs[:, h * 64:(h + 1) * 64],
                         rhs=sb_id8[:], is_transpose=True, start=(h == 0), stop=False,
                         skip_group_check=True)
        nc.tensor.matmul(den_ps[:, h * 8:(h + 1) * 8], lhsT=expn[:, h:h + 1],
                         rhs=sb_id8[:], is_transpose=True, start=(h == 0), stop=False,
                         skip_group_check=True)

    def pair_iter(pb):
        b0, b1 = 2 * pb, 2 * pb + 1
        krot = krpool.tile([128, CL], F16, tag="kr")
        kpas = kppool.tile([128, CL], F16, tag="kp")
        vt0 = vpool.tile([128, CL], F8, tag="v", name=f"v{b0}")
        vt1 = vpool.tile([128, CL], F8, tag="v", name=f"v{b1}")
        # b0 parts on SP, b1 parts on Act: the two queues stream concurrently
        nc.sync.dma_start(krot[0:64, :], k_c[b0][0:64])
        nc.scalar.dma_start(krot[64:128, :], k_c[b1][0:64])
        nc.sync.dma_start(kpas[0:64, :], k_c[b0][64:128])
        nc.scalar.dma_start(kpas[64:128, :], k_c[b1][64:128])
        nc.sync.dma_start(vt0[:], v_c[b0])
        nc.scalar.dma_start(vt1[:], v_c[b1])
        if pb == 0:
            # second table halves ride the SP queue behind pair-0's K
            nc.sync.dma_start(sb_cos[:, HALF:CL], t["cos_t"][:, HALF:CL])
            nc.sync.dma_start(sb_sin[:, HALF:CL], t["sin_t"][:, HALF:CL])

        # paired rot products in col-halves; engine pick via PAIR_POOL knob
        kcr = kcpool.tile([128, CL], F16, tag="kc")
        ksr = kspool.tile([128, CL], F16, tag="ks")
        sc0 = psum_sc.tile([128, 64], F32, tag="sc", name=f"sc{b0}")
        sc1 = psum_sc.tile([128, 64], F32, tag="sc", name=f"sc{b1}")
        for half in range(2):
            lo, hi = half * HALF, (half + 1) * HALF
            kc_eng = nc.gpsimd if (pb, half, 0) in PAIR_POOL else nc.vector
            ks_eng = nc.gpsimd if (pb, half, 1) in PAIR_POOL else nc.vector
            kc_eng.tensor_mul(kcr[:, lo:hi], krot[:, lo:hi], sb_cos[:, lo:hi])
            ks_eng.tensor_mul(ksr[:, lo:hi], krot[:, lo:hi], sb_sin[:, lo:hi])
            for c in range(half * 16, half * 16 + 16):
                cs = slice(c * 128, (c + 1) * 128)
                for b, sc in ((b0, sc0), (b1, sc1)):
                    nc.tensor.matmul(sc[:, 2 * c:2 * c + 2], lhsT=kcr[:, cs],
                                     rhs=um[:, 2 * b:2 * b + 2],
                                     start=(c == 0), stop=False,
                                     skip_group_check=True)
                    nc.tensor.matmul(sc[:, 2 * c:2 * c + 2], lhsT=ksr[:, cs],
                                     rhs=vm[:, 2 * b:2 * b + 2],
                                     start=False, stop=False,
                                     skip_group_check=True)
                    nc.tensor.matmul(sc[:, 2 * c:2 * c + 2], lhsT=kpas[:, cs],
                                     rhs=upm[:, 2 * b:2 * b + 2],
                                     start=False, stop=(c == 31),
                                     skip_group_check=True)

        # exp + denominators + A.V per batch of the pair
        for b, sc, vt in ((b0, sc0, vt0), (b1, sc1, vt1)):
            at = apool.tile([128, 64], F16, tag="at")
            for h in range(H_PER_CORE):
                col = h * 8 + b
                scv = _fap(sc, h, [[2, 32]])
                nc.scalar.activation(at[:, h * 32:(h + 1) * 32], scv,
                                     AF.Exp, scale=0.125,
                                     accum_out=den_part[:, col:col + 1])
                for c in range(32):
                    nc.tensor.matmul(ov_ps[:, col:col + 1],
                                     lhsT=_fap(vt, h * 2048 + c * 64, [[1, 64]]),
                                     rhs=at[:, h * 32 + c:h * 32 + c + 1],
                                     start=False, stop=(c == 31),
                                     skip_group_check=True)

    for pb in range(4):
        pair_iter(pb)

    # denominator: column-sum of per-partition exp sums + new-token init
    nc.tensor.matmul(den_ps[:], lhsT=ones_p[:], rhs=den_part[:],
                     start=False, stop=True, skip_group_check=True)

    # ---- normalize + out-projection
    ov_sb = small.tile([64, 16], F32, tag="ov_sb")
    nc.scalar.copy(ov_sb[:], ov_ps[:])
    r_row = small.tile([1, 16], F32, tag="r_row")
    nc.vector.reciprocal(r_row[:], den_ps[:])
    r_ps = psum_r.tile([64, 16], F32, tag="r")
    nc.tensor.matmul(r_ps[:], lhsT=ones_r64[:], rhs=r_row[:], start=True, stop=True)
    on = small.tile([64, 16], F16, tag="on")
    nc.vector.tensor_mul(on[:], ov_sb[:], r_ps[:])

    out_f = small.tile([8, 1024], F32, tag="out_f")
    for nchunk in range(2):
        sl = slice(nchunk * 512, (nchunk + 1) * 512)
        ps = psum_wo.tile([8, 512], F32, tag="wo", name=f"wo_ps{nchunk}")
        nc.tensor.matmul(ps[:], lhsT=on[:, 0:8], rhs=sb_wo0[:, sl], start=True, stop=False)
        nc.tensor.matmul(ps[:], lhsT=on[:, 8:16], rhs=sb_wo1[:, sl], start=False, stop=True)
        if nchunk == 0:
            nc.vector.tensor_copy(out_f[:, sl], ps[:])
        else:
            nc.scalar.copy(out_f[:, sl], ps[:])
        (nc.sync if nchunk == 0 else nc.scalar).dma_start(out_p[:, sl], out_f[:, sl])


def _host_tables():
    """cos~/sin~ in transposed layout [128 (h,d), 4096 s] plus q-side tables."""
    inv_freq = 1.0 / (THETA ** (np.arange(0, ROT, 2, dtype=np.float64) / ROT))
    invf_rep = np.repeat(inv_freq, 2)  # [32]
    pos = np.arange(CL, dtype=np.float64)
    ang = invf_rep[:, None] * pos[None, :]  # [32 rot-d, 4096 s]
    # rows (pair-slot, h, j): the same 32 rotary rows tiled 4x
    cos_t = np.tile(np.cos(ang), (4, 1)).astype(np.float16)  # [128, 4096]
    sin_t = np.tile(np.sin(ang), (4, 1)).astype(np.float16)
    fq = 4096.0 * invf_rep
    cq_row = np.concatenate([np.cos(fq), np.ones(32)])  # per head [64]
    sq_row = np.concatenate([np.sin(fq), np.zeros(32)])
    cq_t = np.tile(np.concatenate([cq_row, cq_row]), (BS, 1)).astype(np.float32)
    sq_t = np.tile(np.concatenate([sq_row, sq_row]), (BS, 1)).astype(np.float32)
    return cos_t, sin_t, cq_t, sq_t


_NC = None


def _get_nc():
    global _NC
    if _NC is None:
        _NC = build_program()
    return _NC


def kernel(q, k_cache, v_cache, WQ_w, WQ_b, WK_w, WK_b, WV_w, WV_b, WO_w, WO_b,
           _trace=False, _tmpdir=None):
    q = np.asarray(q, dtype=np.float32)
    k16 = np.asarray(k_cache, dtype=np.float32).astype(np.float16)
    v8 = np.asarray(v_cache, dtype=np.float32).astype(ml_dtypes.float8_e3m4)
    cos_t, sin_t, cq_t, sq_t = _host_tables()
    q_t = np.ascontiguousarray(q.reshape(BS, D).T.astype(np.float16))
    id8 = np.eye(8, dtype=np.float32)
    id8f = np.eye(8, dtype=np.float16)
    # pair-slot head selectors over (slot, h, d') rows
    hsel0 = np.zeros((128, 2), np.float16)
    hsel0[0:32, 0] = 1.0
    hsel0[32:64, 1] = 1.0
    hsel1 = np.zeros((128, 2), np.float16)
    hsel1[64:96, 0] = 1.0
    hsel1[96:128, 1] = 1.0

    in_maps = []
    for c in range(N_CORES):
        sl = slice(c * 128, (c + 1) * 128)
        hs = slice(c * H_PER_CORE, (c + 1) * H_PER_CORE)
        # K: [b,h,s,d] -> [b, (rot|pass, h, d'), s]
        kk = k16[:, hs].transpose(0, 1, 3, 2)  # [b, h, d, s]
        kc = np.concatenate([kk[:, :, 0:32], kk[:, :, 32:64]],
                            axis=1).reshape(BS, 128, CL)
        # V: [b,h,s,d] -> [b, p, (h c d)] with s = c*128 + p
        vc = v8[:, hs].reshape(BS, H_PER_CORE, 32, 128, HD)
        vc = vc.transpose(0, 3, 1, 2, 4).reshape(BS, 128, CL)
        in_maps.append({
            "k_c": np.ascontiguousarray(kc),
            "v_c": np.ascontiguousarray(vc),
            "q_t": q_t,
            "wqkv_t": np.ascontiguousarray(np.concatenate(
                [np.asarray(WQ_w, np.float32)[sl].T,
                 np.asarray(WK_w, np.float32)[sl].T,
                 np.asarray(WV_w, np.float32)[sl].T], axis=1).astype(np.float16)),
            "bqkv": np.ascontiguousarray(np.concatenate(
                [np.asarray(WQ_b, np.float32)[sl],
                 np.asarray(WK_b, np.float32)[sl],
                 np.asarray(WV_b, np.float32)[sl]]).reshape(1, 384).astype(np.float16)),
            "wo_t": np.ascontiguousarray(
                np.asarray(WO_w, np.float32)[:, sl].T.astype(np.float16)),
            "cos_t": cos_t, "sin_t": sin_t, "cq_t": cq_t, "sq_t": sq_t,
            "id8": id8, "id8f": id8f, "hsel0": hsel0, "hsel1": hsel1,
        })

    nc = _get_nc()
    res = run_bass_kernel_spmd(nc, in_maps, list(range(N_CORES)),
                               trace=_trace, tmpdir=_tmpdir)
    partials = [np.asarray(res.results[c]["out_p"], dtype=np.float64)
                for c in range(N_CORES)]
    out = np.sum(partials, axis=0) + np.asarray(WO_b, np.float64)
    if _trace:
        kernel._last_results = res
    return out.reshape(BS, 1, D).astype(np.float32)


# revision 26
# speedup vs baseline: 1.0510x; 1.0510x over previous
"""Trainium2 Bass kernel for decode-step multi-head attention with RoPE
re-applied to the full KV cache (nn_MultiHeadAttention_50216757624897).

Sharding: 16 heads tensor-parallel across 8 cores (2 heads/core).
QKV weights split column-wise by head, KV cache split on the head dim,
out-proj row-parallel; partials summed on host (the unshard step).

Architecture (v4, transposed-K, batch-paired rot products, 3-pass fold):
 - K cache host-permuted to [b, (rot|pass, h, d'), s] fp16: partitions are
   the 2x32 rotary dims then the 2x32 passthrough dims; free dim carries
   all 4096 positions.
 - score[s,h] = sum_rot k*cos*u + sum_rot k*sin*v + sum_pass k*u. The
   passthrough sum needs NO elementwise work at all (k itself is the
   stationary operand, u folds into the mask); the rotary halves of TWO
   batches pack into one 128-partition tile, so kcr = krot (.) cos and
   ksr = krot (.) sin each cover a batch PAIR in one DVE-2x/Pool op.
   Per 128-position chunk, three accumulating matmuls per batch (kcr/um,
   ksr/vm, kpass/upm masks) write scores [128 pos, 2 heads] to PSUM.
 - u/v/u_pass per-batch mask columns are built from a tiny DRAM bounce of
   the transposed q (rot rows duplicated into both pair slots).
 - The new (current) token's K rotation cancels with Q's: score_new = qh.kh.
 - Softmax runs without max-subtraction (|score/8| < 3).
 - V cache host-permuted to [b, p, (h, c, d)] fp8-e3m4 (position = c*128+p):
   it is consumed only by PE A.V matmuls (fp8 full-rate), halving its HBM
   traffic; A.V contracts over partitions like the score layout.
 - DMA transfers overlap across issuing queues in the cost model; the kv
   stream alternates SP/Act and the cos/sin tables ship as halves on
   SWDGE/Act so both tables land by ~6us.
"""

import os
import sys
from contextlib import ExitStack

import numpy as np
import ml_dtypes

sys.path.insert(0, "/opt/trn_rl_repo")

import concourse.bass as bass
import concourse.bacc as bacc
import concourse.tile as tile
from concourse import mybir
from concourse.bass_types import AP
from concourse.bass_utils import run_bass_kernel_spmd

F32 = mybir.dt.float32
F16 = mybir.dt.float16
F8 = mybir.dt.float8e3
AF = mybir.ActivationFunctionType
AX = mybir.AxisListType
OP = mybir.AluOpType

BS, NH, HD, ROT, CL, D = 8, 16, 64, 32, 4096, 1024
THETA = 10000.0
N_CORES = 8
H_PER_CORE = NH // N_CORES  # 2
HALF = CL // 2

# (pair, col-half, product[0=kc,1=ks]) triples whose multiply runs on Pool
def _parse_pp(s):
    out = set()
    for item in s.split(";"):
        if item:
            a, b, c = item.split(",")
            out.add((int(a), int(b), int(c)))
    return out

PAIR_POOL = _parse_pp(os.environ.get(
    "PAIR_POOL", "0,0,1;0,1,1;1,0,1;1,1,1;2,0,1;2,1,1"))


def _fap(t, off, dims):
    """AP over tile t with the tile's partition dim, extra free-dim spec."""
    b = t[:]
    return AP(tensor=b.tensor, offset=b.offset + off, ap=[list(b.ap[0])] + dims)


def _rotap(t, off):
    """[8, 2h, 16pairs] strided view of a [8,128] tile selecting pair elem
    `off` (0=even, 1=odd) of the rotary dims."""
    return _fap(t, off, [[64, 2], [2, 16]])


def build_program():
    nc = bacc.Bacc("TRN2", target_bir_lowering=False, debug=False)

    k_c = nc.dram_tensor("k_c", [BS // 2, 2, 128, CL], F16, kind="ExternalInput")
    v_c = nc.dram_tensor("v_c", [BS, 128, CL], F8, kind="ExternalInput")
    q_t = nc.dram_tensor("q_t", [D, BS], F16, kind="ExternalInput")
    wqkv_t = nc.dram_tensor("wqkv_t", [D, 384], F16, kind="ExternalInput")
    bqkv = nc.dram_tensor("bqkv", [1, 384], F16, kind="ExternalInput")
    wo_t = nc.dram_tensor("wo_t", [128, D], F16, kind="ExternalInput")
    cos_t = nc.dram_tensor("cos_t", [128, CL], F16, kind="ExternalInput")
    sin_t = nc.dram_tensor("sin_t", [128, CL], F16, kind="ExternalInput")
    cq_t = nc.dram_tensor("cq_t", [BS, 128], F32, kind="ExternalInput")
    sq_t = nc.dram_tensor("sq_t", [BS, 128], F32, kind="ExternalInput")
    id8 = nc.dram_tensor("id8", [8, 8], F32, kind="ExternalInput")
    id8f = nc.dram_tensor("id8f", [8, 8], F16, kind="ExternalInput")
    hsel0 = nc.dram_tensor("hsel0", [128, 2], F16, kind="ExternalInput")
    hsel1 = nc.dram_tensor("hsel1", [128, 2], F16, kind="ExternalInput")
    out_p = nc.dram_tensor("out_p", [BS, D], F32, kind="ExternalOutput")

    with tile.TileContext(nc) as tc:
        with ExitStack() as ctx:
            _body(nc, tc, ctx, locals())
    nc.finalize()
    return nc


def _body(nc, tc, ctx, t):
    k_c, v_c = t["k_c"], t["v_c"]
    out_p = t["out_p"]

    const = ctx.enter_context(tc.tile_pool(name="const", bufs=1))
    small = ctx.enter_context(tc.tile_pool(name="small", bufs=1))

    # ---- constants. q/wqkv gate the q-chain (SP); table halves spread over
    # SWDGE + Act so both tables land by ~6us; kv stream follows.
    sb_qt = const.tile([128, 8, 8], F16, tag="qt")
    nc.sync.dma_start(sb_qt[:], t["q_t"].rearrange("(c p) b -> p c b", p=128))
    sb_wqkv = const.tile([128, 8, 384], F16, tag="wqkv")
    nc.sync.dma_start(sb_wqkv[:], t["wqkv_t"].rearrange("(c p) n -> p c n", p=128))

    sb_cos = const.tile([128, CL], F16, tag="cos")
    sb_sin = const.tile([128, CL], F16, tag="sin")
    nc.gpsimd.dma_start(sb_cos[:, 0:HALF], t["cos_t"][:, 0:HALF])
    nc.gpsimd.dma_start(sb_sin[:, 0:HALF], t["sin_t"][:, 0:HALF])

    # batch-0 K/V jump the Act queue so kc_0 can start when the tables land
    kt0 = None  # placeholder; the loop below reuses tiles allocated here
    sb_bqkv = const.tile([1, 384], F16, tag="bqkv")
    nc.scalar.dma_start(sb_bqkv[:], t["bqkv"][:, :])
    sb_cq = const.tile([BS, 128], F32, tag="cq")
    nc.scalar.dma_start(sb_cq[:], t["cq_t"][:, :])
    sb_sq = const.tile([BS, 128], F32, tag="sq")
    nc.scalar.dma_start(sb_sq[:], t["sq_t"][:, :])
    sb_id8 = const.tile([8, 8], F32, tag="id8")
    nc.scalar.dma_start(sb_id8[:], t["id8"][:, :])
    sb_id8f = const.tile([8, 8], F16, tag="id8f")
    nc.scalar.dma_start(sb_id8f[:], t["id8f"][:, :])
    sb_hsel0 = const.tile([128, 2], F16, tag="hsel0")
    nc.scalar.dma_start(sb_hsel0[:], t["hsel0"][:, :])
    sb_hsel1 = const.tile([128, 2], F16, tag="hsel1")
    nc.scalar.dma_start(sb_hsel1[:], t["hsel1"][:, :])
    sb_wo0 = const.tile([64, 1024], F16, tag="wo0")
    nc.scalar.dma_start(sb_wo0[:], t["wo_t"][0:64, :])
    sb_wo1 = const.tile([64, 1024], F16, tag="wo1")
    nc.scalar.dma_start(sb_wo1[:], t["wo_t"][64:128, :])

    ones_p = const.tile([128, 1], F32, tag="ones_p")
    nc.vector.memset(ones_p[:], 1.0)
    ones_r8 = const.tile([1, 8], F16, tag="ones_r8")
    nc.vector.memset(ones_r8[:], 1.0)
    ones_r64 = const.tile([1, 64], F32, tag="ones_r64")
    nc.vector.memset(ones_r64[:], 1.0)

    # ---- q/k/v projection of the new token
    qtr_stack = ExitStack()
    psum_proj = qtr_stack.enter_context(tc.tile_pool(name="psum_proj", bufs=1, space="PSUM"))
    projs = small.tile([8, 384], F32, tag="projs")
    ps_qkv = psum_proj.tile([8, 384], F32, tag="ps_qkv")
    ps_q = ps_qkv[:, 0:128]
    for ci in range(8):
        nc.tensor.matmul(ps_q, lhsT=sb_qt[:, ci, :], rhs=sb_wqkv[:, ci, 0:128],
                         start=(ci == 0), stop=False, skip_group_check=True)
    nc.tensor.matmul(ps_q, lhsT=ones_r8[:], rhs=sb_bqkv[:, 0:128],
                     start=False, stop=True, skip_group_check=True)
    nc.scalar.copy(projs[:, 0:128], ps_q)
    ps_kv = ps_qkv[:, 128:384]
    for ci in range(8):
        nc.tensor.matmul(ps_kv, lhsT=sb_qt[:, ci, :], rhs=sb_wqkv[:, ci, 128:384],
                         start=False, stop=False, skip_group_check=True)
    nc.tensor.matmul(ps_kv, lhsT=ones_r8[:], rhs=sb_bqkv[:, 128:384],
                     start=False, stop=True, skip_group_check=True)
    nc.scalar.copy(projs[:, 128:384], ps_kv)
    qh, kh, vh = projs[:, 0:128], projs[:, 128:256], projs[:, 256:384]

    # ---- RoPE on q (full width: host tables carry [cos|1], [sin|0]); u and
    # v = G(u) side by side in one [8, 256] f16 tile.
    qrv = small.tile([8, 256], F16, tag="qrv")
    qr, vG = qrv[:, 0:128], qrv[:, 128:256]
    Hh = small.tile([8, 128], F32, tag="Hh")
    nc.vector.memset(Hh[:], 0.0)
    nc.vector.tensor_scalar_mul(_rotap(Hh, 0), _fap(ps_qkv, 1, [[64, 2], [2, 16]]), -1.0)
    nc.vector.tensor_copy(_rotap(Hh, 1), _fap(ps_qkv, 0, [[64, 2], [2, 16]]))
    t1q = small.tile([8, 128], F32, tag="t1q")
    nc.vector.tensor_mul(t1q[:], ps_q, sb_cq[:])
    t2q = small.tile([8, 128], F32, tag="t2q")
    nc.vector.tensor_mul(t2q[:], Hh[:], sb_sq[:])
    nc.vector.tensor_add(qr, t2q[:], t1q[:])
    # v = G(q_rot): pairs (x0,x1) -> (x1,-x0); zero elsewhere
    nc.vector.memset(vG, 0.0)
    nc.vector.tensor_copy(_fap(qrv, 128, [[64, 2], [2, 16]]),
                          _fap(qrv, 1, [[64, 2], [2, 16]]))
    nc.vector.tensor_scalar_mul(_fap(qrv, 129, [[64, 2], [2, 16]]),
                                _fap(qrv, 0, [[64, 2], [2, 16]]), -1.0)

    # ---- transpose u, v to [128 (rot|pass, h, d'), 8 b]: lhsT views reorder
    # the free dims so rotary rows land first
    psum_tr = qtr_stack.enter_context(tc.tile_pool(name="psum_tr", bufs=1, space="PSUM"))
    qb = qrv[:]
    qr_ro = AP(tensor=qb.tensor, offset=qb.offset,
               ap=[list(qb.ap[0]), [32, 2], [64, 2], [1, 32]])
    vG_ro = AP(tensor=qb.tensor, offset=qb.offset + 128,
               ap=[list(qb.ap[0]), [32, 2], [64, 2], [1, 32]])
    qro = small.tile([8, 256], F16, tag="qro")
    nc.vector.tensor_copy(qro[:, 0:128], qr_ro)
    nc.vector.tensor_copy(qro[:, 128:256], vG_ro)
    uv_ps = psum_tr.tile([128, 16], F16, tag="uv_ps")
    nc.tensor.matmul(uv_ps[:, 0:8], lhsT=qro[:, 0:128], rhs=sb_id8f[:],
                     is_transpose=True, start=True, stop=False,
                     skip_group_check=True)
    nc.tensor.matmul(uv_ps[:, 8:16], lhsT=qro[:, 128:256], rhs=sb_id8f[:],
                     is_transpose=True, start=False, stop=True,
                     skip_group_check=True)
    uv_T = small.tile([128, 16], F32, tag="uv_T")
    nc.scalar.copy(uv_T[:], uv_ps[:])

    # bounce through DRAM to duplicate the rot/pass row groups into both
    # pair slots: urot2/vrot2/upass2 [128, 8] with rows [grp | grp]
    uvd = nc.dram_tensor("uv_scratch", [128, 16], F32, kind="Internal")
    uvb = uv_T[:]
    nc.sync.dma_start(uvd[:, :], uv_T[:])
    dup = small.tile([128, 24], F32, tag="dup")
    dsrc = uvd[:, :]
    # urot2: dram rows 0:64 (u rot) duplicated
    nc.sync.dma_start(dup[:, 0:8], AP(tensor=dsrc.tensor, offset=dsrc.offset,
                                      ap=[[0, 2], [16, 64], [1, 8]]))
    # vrot2: dram rows 0:64 of the v half (offset 8)
    nc.sync.dma_start(dup[:, 8:16], AP(tensor=dsrc.tensor, offset=dsrc.offset + 8,
                                       ap=[[0, 2], [16, 64], [1, 8]]))
    # upass2: dram rows 64:128 of u
    nc.sync.dma_start(dup[:, 16:24], AP(tensor=dsrc.tensor, offset=dsrc.offset + 64 * 16,
                                        ap=[[0, 2], [16, 64], [1, 8]]))

    # per-batch mask columns [128, 2]: um/vm on the rot rows of the batch's
    # pair slot, upm on the pass rows
    um = small.tile([128, 16], F16, tag="um")
    vm = small.tile([128, 16], F16, tag="vm")
    upm = small.tile([128, 16], F16, tag="upm")
    for b in range(8):
        hsel = sb_hsel0 if b % 2 == 0 else sb_hsel1
        nc.vector.tensor_scalar(um[:, 2 * b:2 * b + 2], hsel[:],
                                dup[:, b:b + 1], None, OP.mult)
        nc.vector.tensor_scalar(vm[:, 2 * b:2 * b + 2], hsel[:],
                                dup[:, 8 + b:8 + b + 1], None, OP.mult)
        nc.vector.tensor_scalar(upm[:, 2 * b:2 * b + 2], hsel[:],
                                dup[:, 16 + b:16 + b + 1], None, OP.mult)

    # ---- new-token score: rotations cancel -> qh . kh
    sn = small.tile([8, 128], F32, tag="sn")
    nc.vector.tensor_mul(sn[:], qh, kh)
    scn = small.tile([8, 2], F32, tag="scn")
    nc.vector.reduce_sum(scn[:], _fap(sn, 0, [[64, 2], [1, 64]]), axis=AX.X)
    expn = small.tile([8, 2], F32, tag="expn")
    nc.scalar.activation(expn[:], scn[:], AF.Exp, scale=0.125)
    vhs = small.tile([8, 128], F32, tag="vhs")
    nc.vector.tensor_mul(_fap(vhs, 0, [[64, 2], [1, 64]]),
                         _fap(projs, 256, [[64, 2], [1, 64]]),
                         _fap(expn, 0, [[1, 2], [0, 64]]))

    qtr_stack.close()  # release proj/transpose PSUM banks for the loop pools

    # ---- main per-pair loop
    krpool = ctx.enter_context(tc.tile_pool(name="krpool", bufs=2))
    kppool = ctx.enter_context(tc.tile_pool(name="kppool", bufs=2))
    vpool = ctx.enter_context(tc.tile_pool(name="vpool", bufs=4))
    kcpool = ctx.enter_context(tc.tile_pool(name="kcpool", bufs=2))
    kspool = ctx.enter_context(tc.tile_pool(name="kspool", bufs=2))
    apool = ctx.enter_context(tc.tile_pool(name="apool", bufs=3))
    psum_sc = ctx.enter_context(tc.tile_pool(name="psum_sc", bufs=3, space="PSUM"))
    psum_r = ctx.enter_context(tc.tile_pool(name="psum_r", bufs=1, space="PSUM"))
    psum_wo = ctx.enter_context(tc.tile_pool(name="psum_wo", bufs=2, space="PSUM"))
    psum_main = ctx.enter_context(tc.tile_pool(name="psum_main", bufs=1, space="PSUM"))

    ov_ps = psum_main.tile([64, 16], F32, tag="ov")
    den_ps = psum_main.tile([1, 16], F32, tag="den")
    den_part = small.tile([128, 16], F32, tag="den_part")

    # init PSUM with the new-token contribution (transposes of vh*exp, exp)
    # NOTE: PSUM start=True zeroes the whole 2KB bank row, so only the FIRST
    # write into each psum tile may use start=True.
    for h in range(H_PER_CORE):
        nc.tensor.matmul(ov_ps[:, h * 8:(h + 1) * 8], lhsT=vhs[:, h * 64:(h + 1) * 64],
                         rhs=sb_id8[:], is_transpose=True, start=(h == 0), stop=False,
                         skip_group_check=True)
        nc.tensor.matmul(den_ps[:, h * 8:(h + 1) * 8], lhsT=expn[:, h:h + 1],
                         rhs=sb_id8[:], is_transpose=True, start=(h == 0), stop=False,
                         skip_group_check=True)

    def pair_iter(pb):
        b0, b1 = 2 * pb, 2 * pb + 1
        krot = krpool.tile([128, CL], F16, tag="kr")
        kpas = kppool.tile([128, CL], F16, tag="kp")
        vt0 = vpool.tile([128, CL], F8, tag="v", name=f"v{b0}")
        vt1 = vpool.tile([128, CL], F8, tag="v", name=f"v{b1}")
        # krot gates compute: alternate it between the two queues per pair
        qa, qb_ = (nc.sync, nc.scalar) if pb % 2 == 0 else (nc.scalar, nc.sync)
        qa.dma_start(krot[:], k_c[pb, 0])
        if pb == 0:
            # second table halves follow pair-0's krot on the other queue
            nc.scalar.dma_start(sb_cos[:, HALF:CL], t["cos_t"][:, HALF:CL])
            nc.scalar.dma_start(sb_sin[:, HALF:CL], t["sin_t"][:, HALF:CL])
        qb_.dma_start(kpas[:], k_c[pb, 1])
        qa.dma_start(vt0[:], v_c[b0])
        qb_.dma_start(vt1[:], v_c[b1])

        # paired rot products in col-halves; engine pick via PAIR_POOL knob
        kcr = kcpool.tile([128, CL], F16, tag="kc")
        ksr = kspool.tile([128, CL], F16, tag="ks")
        sc0 = psum_sc.tile([128, 64], F32, tag="sc", name=f"sc{b0}")
        sc1 = psum_sc.tile([128, 64], F32, tag="sc", name=f"sc{b1}")
        for half in range(2):
            lo, hi = half * HALF, (half + 1) * HALF
            kc_eng = nc.gpsimd if (pb, half, 0) in PAIR_POOL else nc.vector
            ks_eng = nc.gpsimd if (pb, half, 1) in PAIR_POOL else nc.vector
            kc_eng.tensor_mul(kcr[:, lo:hi], krot[:, lo:hi], sb_cos[:, lo:hi])
            ks_eng.tensor_mul(ksr[:, lo:hi], krot[:, lo:hi], sb_sin[:, lo:hi])
            for c in range(half * 16, half * 16 + 16):
                cs = slice(c * 128, (c + 1) * 128)
                for b, sc in ((b0, sc0), (b1, sc1)):
                    nc.tensor.matmul(sc[:, 2 * c:2 * c + 2], lhsT=kcr[:, cs],
                                     rhs=um[:, 2 * b:2 * b + 2],
                                     start=(c == 0), stop=False,
                                     skip_group_check=True)
                    nc.tensor.matmul(sc[:, 2 * c:2 * c + 2], lhsT=ksr[:, cs],
                                     rhs=vm[:, 2 * b:2 * b + 2],
                                     start=False, stop=False,
                                     skip_group_check=True)
                    nc.tensor.matmul(sc[:, 2 * c:2 * c + 2], lhsT=kpas[:, cs],
                                     rhs=upm[:, 2 * b:2 * b + 2],
                                     start=False, stop=(c == 31),
                                     skip_group_check=True)

        # exp + denominators + A.V per batch of the pair
        for b, sc, vt in ((b0, sc0, vt0), (b1, sc1, vt1)):
            at = apool.tile([128, 64], F16, tag="at")
            for h in range(H_PER_CORE):
                col = h * 8 + b
                scv = _fap(sc, h, [[2, 32]])
                nc.scalar.activation(at[:, h * 32:(h + 1) * 32], scv,
                                     AF.Exp, scale=0.125,
                                     accum_out=den_part[:, col:col + 1])
                for c in range(32):
                    nc.tensor.matmul(ov_ps[:, col:col + 1],
                                     lhsT=_fap(vt, h * 2048 + c * 64, [[1, 64]]),
                                     rhs=at[:, h * 32 + c:h * 32 + c + 1],
                                     start=False, stop=(c == 31),
                                     skip_group_check=True)

    for pb in range(4):
        pair_iter(pb)

    # denominator: column-sum of per-partition exp sums + new-token init
    nc.tensor.matmul(den_ps[:], lhsT=ones_p[:], rhs=den_part[:],
                     start=False, stop=True, skip_group_check=True)

    # ---- normalize + out-projection
    ov_sb = small.tile([64, 16], F32, tag="ov_sb")
    nc.scalar.copy(ov_sb[:], ov_ps[:])
    r_row = small.tile([1, 16], F32, tag="r_row")
    nc.vector.reciprocal(r_row[:], den_ps[:])
    r_ps = psum_r.tile([64, 16], F32, tag="r")
    nc.tensor.matmul(r_ps[:], lhsT=ones_r64[:], rhs=r_row[:], start=True, stop=True)
    on = small.tile([64, 16], F16, tag="on")
    nc.vector.tensor_mul(on[:], ov_sb[:], r_ps[:])

    out_f = small.tile([8, 1024], F32, tag="out_f")
    for nchunk in range(2):
        sl = slice(nchunk * 512, (nchunk + 1) * 512)
        ps = psum_wo.tile([8, 512], F32, tag="wo", name=f"wo_ps{nchunk}")
        nc.tensor.matmul(ps[:], lhsT=on[:, 0:8], rhs=sb_wo0[:, sl], start=True, stop=False)
        nc.tensor.matmul(ps[:], lhsT=on[:, 8:16], rhs=sb_wo1[:, sl], start=False, stop=True)
        if nchunk == 0:
            nc.vector.tensor_copy(out_f[:, sl], ps[:])
        else:
            nc.scalar.copy(out_f[:, sl], ps[:])
        (nc.sync if nchunk == 0 else nc.scalar).dma_start(out_p[:, sl], out_f[:, sl])


def _host_tables():
    """cos~/sin~ in transposed layout [128 (h,d), 4096 s] plus q-side tables."""
    inv_freq = 1.0 / (THETA ** (np.arange(0, ROT, 2, dtype=np.float64) / ROT))
    invf_rep = np.repeat(inv_freq, 2)  # [32]
    pos = np.arange(CL, dtype=np.float64)
    ang = invf_rep[:, None] * pos[None, :]  # [32 rot-d, 4096 s]
    # rows (pair-slot, h, j): the same 32 rotary rows tiled 4x
    cos_t = np.tile(np.cos(ang), (4, 1)).astype(np.float16)  # [128, 4096]
    sin_t = np.tile(np.sin(ang), (4, 1)).astype(np.float16)
    fq = 4096.0 * invf_rep
    cq_row = np.concatenate([np.cos(fq), np.ones(32)])  # per head [64]
    sq_row = np.concatenate([np.sin(fq), np.zeros(32)])
    cq_t = np.tile(np.concatenate([cq_row, cq_row]), (BS, 1)).astype(np.float32)
    sq_t = np.tile(np.concatenate([sq_row, sq_row]), (BS, 1)).astype(np.float32)
    return cos_t, sin_t, cq_t, sq_t


_NC = None


def _get_nc():
    global _NC
    if _NC is None:
        _NC = build_program()
    return _NC


def kernel(q, k_cache, v_cache, WQ_w, WQ_b, WK_w, WK_b, WV_w, WV_b, WO_w, WO_b,
           _trace=False, _tmpdir=None):
    q = np.asarray(q, dtype=np.float32)
    k16 = np.asarray(k_cache, dtype=np.float32).astype(np.float16)
    v8 = np.asarray(v_cache, dtype=np.float32).astype(ml_dtypes.float8_e3m4)
    cos_t, sin_t, cq_t, sq_t = _host_tables()
    q_t = np.ascontiguousarray(q.reshape(BS, D).T.astype(np.float16))
    id8 = np.eye(8, dtype=np.float32)
    id8f = np.eye(8, dtype=np.float16)
    # pair-slot head selectors over (slot, h, d') rows
    hsel0 = np.zeros((128, 2), np.float16)
    hsel0[0:32, 0] = 1.0
    hsel0[32:64, 1] = 1.0
    hsel1 = np.zeros((128, 2), np.float16)
    hsel1[64:96, 0] = 1.0
    hsel1[96:128, 1] = 1.0

    in_maps = []
    for c in range(N_CORES):
        sl = slice(c * 128, (c + 1) * 128)
        hs = slice(c * H_PER_CORE, (c + 1) * H_PER_CORE)
        # K: [b,h,s,d] -> [pair, rot|pass, (slot, h, d'), s]
        kk = k16[:, hs].transpose(0, 1, 3, 2)  # [b, h, d, s]
        rot = kk[:, :, 0:32].reshape(BS, 64, CL)
        pas = kk[:, :, 32:64].reshape(BS, 64, CL)
        kc = np.empty((BS // 2, 2, 128, CL), np.float16)
        for pb in range(BS // 2):
            kc[pb, 0, 0:64] = rot[2 * pb]
            kc[pb, 0, 64:128] = rot[2 * pb + 1]
            kc[pb, 1, 0:64] = pas[2 * pb]
            kc[pb, 1, 64:128] = pas[2 * pb + 1]
        # V: [b,h,s,d] -> [b, p, (h c d)] with s = c*128 + p
        vc = v8[:, hs].reshape(BS, H_PER_CORE, 32, 128, HD)
        vc = vc.transpose(0, 3, 1, 2, 4).reshape(BS, 128, CL)
        in_maps.append({
            "k_c": np.ascontiguousarray(kc),
            "v_c": np.ascontiguousarray(vc),
            "q_t": q_t,
            "wqkv_t": np.ascontiguousarray(np.concatenate(
                [np.asarray(WQ_w, np.float32)[sl].T,
                 np.asarray(WK_w, np.float32)[sl].T,
                 np.asarray(WV_w, np.float32)[sl].T], axis=1).astype(np.float16)),
            "bqkv": np.ascontiguousarray(np.concatenate(
                [np.asarray(WQ_b, np.float32)[sl],
                 np.asarray(WK_b, np.float32)[sl],
                 np.asarray(WV_b, np.float32)[sl]]).reshape(1, 384).astype(np.float16)),
            "wo_t": np.ascontiguousarray(
                np.asarray(WO_w, np.float32)[:, sl].T.astype(np.float16)),
            "cos_t": cos_t, "sin_t": sin_t, "cq_t": cq_t, "sq_t": sq_t,
            "id8": id8, "id8f": id8f, "hsel0": hsel0, "hsel1": hsel1,
        })

    nc = _get_nc()
    res = run_bass_kernel_spmd(nc, in_maps, list(range(N_CORES)),
                               trace=_trace, tmpdir=_tmpdir)
    partials = [np.asarray(res.results[c]["out_p"], dtype=np.float64)
                for c in range(N_CORES)]
    out = np.sum(partials, axis=0) + np.asarray(WO_b, np.float64)
    if _trace:
        kernel._last_results = res
    return out.reshape(BS, 1, D).astype(np.float32)


# revision 36
# speedup vs baseline: 1.4652x; 1.3941x over previous
"""Trainium2 Bass kernel for decode-step multi-head attention with RoPE
re-applied to the full KV cache (nn_MultiHeadAttention_50216757624897).

Sharding: 16 heads tensor-parallel across 8 cores (2 heads/core).
QKV weights split column-wise by head, KV cache split on the head dim,
out-proj row-parallel; partials summed on host (the unshard step).

Architecture (v4, transposed-K, batch-paired rot products, 3-pass fold):
 - K cache host-permuted to [b, (rot|pass, h, d'), s] fp16: partitions are
   the 2x32 rotary dims then the 2x32 passthrough dims; free dim carries
   all 4096 positions.
 - score[s,h] = sum_rot k*cos*u + sum_rot k*sin*v + sum_pass k*u. The
   passthrough sum needs NO elementwise work at all (k itself is the
   stationary operand, u folds into the mask); the rotary halves of TWO
   batches pack into one 128-partition tile, so kcr = krot (.) cos and
   ksr = krot (.) sin each cover a batch PAIR in one DVE-2x/Pool op.
   Per 128-position chunk, three accumulating matmuls per batch (kcr/um,
   ksr/vm, kpass/upm masks) write scores [128 pos, 2 heads] to PSUM.
 - u/v/u_pass per-batch mask columns are built from a tiny DRAM bounce of
   the transposed q (rot rows duplicated into both pair slots).
 - The new (current) token's K rotation cancels with Q's: score_new = qh.kh.
 - Softmax runs without max-subtraction (|score/8| < 3).
 - V cache host-permuted to [b, p, (h, c, d)] fp8-e3m4 (position = c*128+p):
   it is consumed only by PE A.V matmuls (fp8 full-rate), halving its HBM
   traffic; A.V contracts over partitions like the score layout.
 - DMA transfers overlap across issuing queues in the cost model; the kv
   stream alternates SP/Act and the cos/sin tables ship as halves on
   SWDGE/Act so both tables land by ~6us.
"""

import os
import sys
from contextlib import ExitStack

import numpy as np
import ml_dtypes

sys.path.insert(0, "/opt/trn_rl_repo")

import concourse.bass as bass
import concourse.bacc as bacc
import concourse.tile as tile
from concourse import mybir
from concourse.bass_types import AP
from concourse.bass_utils import run_bass_kernel_spmd

F32 = mybir.dt.float32
F16 = mybir.dt.float16
F8 = mybir.dt.float8e3
AF = mybir.ActivationFunctionType
AX = mybir.AxisListType
OP = mybir.AluOpType

BS, NH, HD, ROT, CL, D = 8, 16, 64, 32, 4096, 1024
THETA = 10000.0
N_CORES = 8
H_PER_CORE = NH // N_CORES  # 2
HALF = CL // 2

# (pair, col-half, product[0=kc,1=ks]) triples whose multiply runs on Pool
def _parse_pp(s):
    out = set()
    for item in s.split(";"):
        if item:
            a, b, c = item.split(",")
            out.add((int(a), int(b), int(c)))
    return out

PAIR_POOL = _parse_pp(os.environ.get(
    "PAIR_POOL", "0,0,1;0,1,1;1,0,1;1,1,1;2,0,1;2,1,1;3,0,1"))


def _fap(t, off, dims):
    """AP over tile t with the tile's partition dim, extra free-dim spec."""
    b = t[:]
    return AP(tensor=b.tensor, offset=b.offset + off, ap=[list(b.ap[0])] + dims)


def _rotap(t, off):
    """[8, 2h, 16pairs] strided view of a [8,128] tile selecting pair elem
    `off` (0=even, 1=odd) of the rotary dims."""
    return _fap(t, off, [[64, 2], [2, 16]])


def build_program():
    nc = bacc.Bacc("TRN2", target_bir_lowering=False, debug=False)

    k_c = nc.dram_tensor("k_c", [BS // 2, 2, 128, CL], F16, kind="ExternalInput")
    v_c = nc.dram_tensor("v_c", [BS, 128, CL], F8, kind="ExternalInput")
    q_t = nc.dram_tensor("q_t", [D, BS], F16, kind="ExternalInput")
    wqkv_t = nc.dram_tensor("wqkv_t", [D, 384], F16, kind="ExternalInput")
    bqkv = nc.dram_tensor("bqkv", [1, 384], F16, kind="ExternalInput")
    wo_t = nc.dram_tensor("wo_t", [128, D], F16, kind="ExternalInput")
    cos_t = nc.dram_tensor("cos_t", [128, CL], F16, kind="ExternalInput")
    sin_t = nc.dram_tensor("sin_t", [128, CL], F16, kind="ExternalInput")
    cq_t = nc.dram_tensor("cq_t", [BS, 128], F32, kind="ExternalInput")
    sq_t = nc.dram_tensor("sq_t", [BS, 128], F32, kind="ExternalInput")
    id8 = nc.dram_tensor("id8", [8, 8], F32, kind="ExternalInput")
    id8f = nc.dram_tensor("id8f", [8, 8], F16, kind="ExternalInput")
    hsel0 = nc.dram_tensor("hsel0", [128, 2], F16, kind="ExternalInput")
    hsel1 = nc.dram_tensor("hsel1", [128, 2], F16, kind="ExternalInput")
    out_p = nc.dram_tensor("out_p", [BS, D], F32, kind="ExternalOutput")

    with tile.TileContext(nc) as tc:
        with ExitStack() as ctx:
            _body(nc, tc, ctx, locals())
    nc.finalize()
    return nc


def _body(nc, tc, ctx, t):
    k_c, v_c = t["k_c"], t["v_c"]
    out_p = t["out_p"]

    const = ctx.enter_context(tc.tile_pool(name="const", bufs=1))
    small = ctx.enter_context(tc.tile_pool(name="small", bufs=1))

    # ---- constants. q/wqkv gate the q-chain (SP); table halves spread over
    # SWDGE + Act so both tables land by ~6us; kv stream follows.
    sb_qt = const.tile([128, 8, 8], F16, tag="qt")
    sb_wqkv = const.tile([128, 8, 384], F16, tag="wqkv")

    sb_cos = const.tile([128, CL], F16, tag="cos")
    sb_sin = const.tile([128, CL], F16, tag="sin")
    nc.gpsimd.dma_start(sb_cos[:, 0:HALF], t["cos_t"][:, 0:HALF])
    nc.gpsimd.dma_start(sb_sin[:, 0:HALF], t["sin_t"][:, 0:HALF])

    # batch-0 K/V jump the Act queue so kc_0 can start when the tables land
    kt0 = None  # placeholder; the loop below reuses tiles allocated here
    sb_bqkv = const.tile([1, 384], F16, tag="bqkv")
    nc.scalar.dma_start(sb_bqkv[:], t["bqkv"][:, :])
    sb_cq = const.tile([BS, 128], F32, tag="cq")
    nc.scalar.dma_start(sb_cq[:], t["cq_t"][:, :])
    sb_sq = const.tile([BS, 128], F32, tag="sq")
    nc.scalar.dma_start(sb_sq[:], t["sq_t"][:, :])
    sb_id8 = const.tile([8, 8], F32, tag="id8")
    nc.scalar.dma_start(sb_id8[:], t["id8"][:, :])
    sb_id8f = const.tile([8, 8], F16, tag="id8f")
    nc.scalar.dma_start(sb_id8f[:], t["id8f"][:, :])
    sb_hsel0 = const.tile([128, 2], F16, tag="hsel0")
    nc.scalar.dma_start(sb_hsel0[:], t["hsel0"][:, :])
    sb_hsel1 = const.tile([128, 2], F16, tag="hsel1")
    nc.scalar.dma_start(sb_hsel1[:], t["hsel1"][:, :])
    sb_wo0 = const.tile([64, 1024], F16, tag="wo0")
    nc.scalar.dma_start(sb_wo0[:], t["wo_t"][0:64, :])
    sb_wo1 = const.tile([64, 1024], F16, tag="wo1")
    nc.scalar.dma_start(sb_wo1[:], t["wo_t"][64:128, :])

    ones_p = const.tile([128, 1], F32, tag="ones_p")
    nc.vector.memset(ones_p[:], 1.0)
    ones_r8 = const.tile([1, 8], F16, tag="ones_r8")
    nc.vector.memset(ones_r8[:], 1.0)
    ones_r64 = const.tile([1, 64], F32, tag="ones_r64")
    nc.vector.memset(ones_r64[:], 1.0)

    # ---- q/k/v projection of the new token
    qtr_stack = ExitStack()
    psum_proj = qtr_stack.enter_context(tc.tile_pool(name="psum_proj", bufs=1, space="PSUM"))
    projs = small.tile([8, 384], F32, tag="projs")
    ps_qkv = psum_proj.tile([8, 384], F32, tag="ps_qkv")
    ps_q = ps_qkv[:, 0:128]
    for ci in range(8):
        nc.tensor.matmul(ps_q, lhsT=sb_qt[:, ci, :], rhs=sb_wqkv[:, ci, 0:128],
                         start=(ci == 0), stop=False, skip_group_check=True)
    nc.tensor.matmul(ps_q, lhsT=ones_r8[:], rhs=sb_bqkv[:, 0:128],
                     start=False, stop=True, skip_group_check=True)
    nc.scalar.copy(projs[:, 0:128], ps_q)
    ps_kv = ps_qkv[:, 128:384]
    for ci in range(8):
        nc.tensor.matmul(ps_kv, lhsT=sb_qt[:, ci, :], rhs=sb_wqkv[:, ci, 128:384],
                         start=False, stop=False, skip_group_check=True)
    nc.tensor.matmul(ps_kv, lhsT=ones_r8[:], rhs=sb_bqkv[:, 128:384],
                     start=False, stop=True, skip_group_check=True)
    nc.scalar.copy(projs[:, 128:384], ps_kv)
    qh, kh, vh = projs[:, 0:128], projs[:, 128:256], projs[:, 256:384]

    # ---- RoPE on q (full width: host tables carry [cos|1], [sin|0]); u and
    # v = G(u) side by side in one [8, 256] f16 tile.
    qrv = small.tile([8, 256], F16, tag="qrv")
    qr, vG = qrv[:, 0:128], qrv[:, 128:256]
    Hh = small.tile([8, 128], F32, tag="Hh")
    nc.vector.memset(Hh[:], 0.0)
    nc.vector.tensor_scalar_mul(_rotap(Hh, 0), _fap(ps_qkv, 1, [[64, 2], [2, 16]]), -1.0)
    nc.vector.tensor_copy(_rotap(Hh, 1), _fap(ps_qkv, 0, [[64, 2], [2, 16]]))
    t1q = small.tile([8, 128], F32, tag="t1q")
    nc.vector.tensor_mul(t1q[:], ps_q, sb_cq[:])
    t2q = small.tile([8, 128], F32, tag="t2q")
    nc.vector.tensor_mul(t2q[:], Hh[:], sb_sq[:])
    nc.vector.tensor_add(qr, t2q[:], t1q[:])
    # v = G(q_rot): pairs (x0,x1) -> (x1,-x0); zero elsewhere
    nc.vector.memset(vG, 0.0)
    nc.vector.tensor_copy(_fap(qrv, 128, [[64, 2], [2, 16]]),
                          _fap(qrv, 1, [[64, 2], [2, 16]]))
    nc.vector.tensor_scalar_mul(_fap(qrv, 129, [[64, 2], [2, 16]]),
                                _fap(qrv, 0, [[64, 2], [2, 16]]), -1.0)

    # ---- transpose u, v to [128 (rot|pass, h, d'), 8 b]: lhsT views reorder
    # the free dims so rotary rows land first
    psum_tr = qtr_stack.enter_context(tc.tile_pool(name="psum_tr", bufs=1, space="PSUM"))
    qb = qrv[:]
    qr_ro = AP(tensor=qb.tensor, offset=qb.offset,
               ap=[list(qb.ap[0]), [32, 2], [64, 2], [1, 32]])
    vG_ro = AP(tensor=qb.tensor, offset=qb.offset + 128,
               ap=[list(qb.ap[0]), [32, 2], [64, 2], [1, 32]])
    qro = small.tile([8, 256], F16, tag="qro")
    nc.vector.tensor_copy(qro[:, 0:128], qr_ro)
    nc.vector.tensor_copy(qro[:, 128:256], vG_ro)
    uv_ps = psum_tr.tile([128, 16], F16, tag="uv_ps")
    nc.tensor.matmul(uv_ps[:, 0:8], lhsT=qro[:, 0:128], rhs=sb_id8f[:],
                     is_transpose=True, start=True, stop=False,
                     skip_group_check=True)
    nc.tensor.matmul(uv_ps[:, 8:16], lhsT=qro[:, 128:256], rhs=sb_id8f[:],
                     is_transpose=True, start=False, stop=True,
                     skip_group_check=True)
    uv_T = small.tile([128, 16], F32, tag="uv_T")
    nc.scalar.copy(uv_T[:], uv_ps[:])

    # bounce through DRAM to duplicate the rot/pass row groups into both
    # pair slots: urot2/vrot2/upass2 [128, 8] with rows [grp | grp]
    uvd = nc.dram_tensor("uv_scratch", [128, 16], F32, kind="Internal")
    uvb = uv_T[:]
    nc.sync.dma_start(uvd[:, :], uv_T[:])
    dup = small.tile([128, 24], F32, tag="dup")
    dsrc = uvd[:, :]
    # urot2: dram rows 0:64 (u rot) duplicated
    nc.sync.dma_start(dup[:, 0:8], AP(tensor=dsrc.tensor, offset=dsrc.offset,
                                      ap=[[0, 2], [16, 64], [1, 8]]))
    # vrot2: dram rows 0:64 of the v half (offset 8)
    nc.sync.dma_start(dup[:, 8:16], AP(tensor=dsrc.tensor, offset=dsrc.offset + 8,
                                       ap=[[0, 2], [16, 64], [1, 8]]))
    # upass2: dram rows 64:128 of u
    nc.sync.dma_start(dup[:, 16:24], AP(tensor=dsrc.tensor, offset=dsrc.offset + 64 * 16,
                                        ap=[[0, 2], [16, 64], [1, 8]]))

    # per-batch mask columns [128, 2]: um/vm on the rot rows of the batch's
    # pair slot, upm on the pass rows
    um = small.tile([128, 16], F16, tag="um")
    vm = small.tile([128, 16], F16, tag="vm")
    upm = small.tile([128, 16], F16, tag="upm")
    for b in range(8):
        hsel = sb_hsel0 if b % 2 == 0 else sb_hsel1
        nc.vector.tensor_scalar(um[:, 2 * b:2 * b + 2], hsel[:],
                                dup[:, b:b + 1], None, OP.mult)
        nc.vector.tensor_scalar(vm[:, 2 * b:2 * b + 2], hsel[:],
                                dup[:, 8 + b:8 + b + 1], None, OP.mult)
        nc.vector.tensor_scalar(upm[:, 2 * b:2 * b + 2], hsel[:],
                                dup[:, 16 + b:16 + b + 1], None, OP.mult)

    # ---- new-token score: rotations cancel -> qh . kh
    sn = small.tile([8, 128], F32, tag="sn")
    nc.vector.tensor_mul(sn[:], qh, kh)
    scn = small.tile([8, 2], F32, tag="scn")
    nc.vector.reduce_sum(scn[:], _fap(sn, 0, [[64, 2], [1, 64]]), axis=AX.X)
    expn = small.tile([8, 2], F32, tag="expn")
    nc.scalar.activation(expn[:], scn[:], AF.Exp, scale=0.125)
    vhs = small.tile([8, 128], F32, tag="vhs")
    nc.vector.tensor_mul(_fap(vhs, 0, [[64, 2], [1, 64]]),
                         _fap(projs, 256, [[64, 2], [1, 64]]),
                         _fap(expn, 0, [[1, 2], [0, 64]]))

    qtr_stack.close()  # release proj/transpose PSUM banks for the loop pools

    # ---- main per-pair loop
    import os as _os
    _bk = int(_os.environ.get("BK", 3))
    _bv = int(_os.environ.get("BV", 6))
    _bc = int(_os.environ.get("BC", 2))
    krpool = ctx.enter_context(tc.tile_pool(name="krpool", bufs=_bk))
    kppool = ctx.enter_context(tc.tile_pool(name="kppool", bufs=int(_os.environ.get("BKP", 3))))
    vpool = ctx.enter_context(tc.tile_pool(name="vpool", bufs=_bv))
    kcpool = ctx.enter_context(tc.tile_pool(name="kcpool", bufs=_bc))
    kspool = ctx.enter_context(tc.tile_pool(name="kspool", bufs=_bc))
    apool = ctx.enter_context(tc.tile_pool(name="apool", bufs=3))
    psum_sc = ctx.enter_context(tc.tile_pool(name="psum_sc", bufs=3, space="PSUM"))
    psum_r = ctx.enter_context(tc.tile_pool(name="psum_r", bufs=1, space="PSUM"))
    psum_wo = ctx.enter_context(tc.tile_pool(name="psum_wo", bufs=2, space="PSUM"))
    psum_main = ctx.enter_context(tc.tile_pool(name="psum_main", bufs=1, space="PSUM"))

    ov_ps = psum_main.tile([64, 16], F32, tag="ov")
    den_ps = psum_main.tile([1, 16], F32, tag="den")
    den_part = small.tile([128, 16], F32, tag="den_part")

    # init PSUM with the new-token contribution (transposes of vh*exp, exp)
    # NOTE: PSUM start=True zeroes the whole 2KB bank row, so only the FIRST
    # write into each psum tile may use start=True.
    for h in range(H_PER_CORE):
        nc.tensor.matmul(ov_ps[:, h * 8:(h + 1) * 8], lhsT=vhs[:, h * 64:(h + 1) * 64],
                         rhs=sb_id8[:], is_transpose=True, start=(h == 0), stop=False,
                         skip_group_check=True)
        nc.tensor.matmul(den_ps[:, h * 8:(h + 1) * 8], lhsT=expn[:, h:h + 1],
                         rhs=sb_id8[:], is_transpose=True, start=(h == 0), stop=False,
                         skip_group_check=True)

    # ---- explicit DMA schedule: three queues stream concurrently; krot
    # tiles (which gate all compute) go first on each queue
    krots = [krpool.tile([128, CL], F16, tag="kr", name=f"kr{i}") for i in range(4)]
    kpass_ = [kppool.tile([128, CL], F16, tag="kp", name=f"kp{i}") for i in range(4)]
    vts = [vpool.tile([128, CL], F8, tag="v", name=f"v{i}") for i in range(8)]

    nc.sync.dma_start(sb_qt[:], t["q_t"].rearrange("(c p) b -> p c b", p=128))
    nc.sync.dma_start(krots[0][:], k_c[0, 0])
    nc.scalar.dma_start(krots[1][:], k_c[1, 0])
    nc.sync.dma_start(sb_wqkv[:], t["wqkv_t"].rearrange("(c p) n -> p c n", p=128))
    nc.sync.dma_start(sb_cos[:, HALF:CL], t["cos_t"][:, HALF:CL])
    nc.sync.dma_start(sb_sin[:, HALF:CL], t["sin_t"][:, HALF:CL])
    nc.scalar.dma_start(kpass_[0][:], k_c[0, 1])
    nc.gpsimd.dma_start(vts[0][:], v_c[0])
    nc.gpsimd.dma_start(vts[1][:], v_c[1])
    nc.sync.dma_start(krots[2][:], k_c[2, 0])
    nc.scalar.dma_start(krots[3][:], k_c[3, 0])
    nc.sync.dma_start(kpass_[1][:], k_c[1, 1])
    nc.scalar.dma_start(kpass_[2][:], k_c[2, 1])
    nc.sync.dma_start(kpass_[3][:], k_c[3, 1])
    nc.gpsimd.dma_start(vts[2][:], v_c[2])
    nc.gpsimd.dma_start(vts[3][:], v_c[3])
    nc.gpsimd.dma_start(vts[4][:], v_c[4])
    nc.gpsimd.dma_start(vts[5][:], v_c[5])
    nc.sync.dma_start(vts[6][:], v_c[6])
    nc.scalar.dma_start(vts[7][:], v_c[7])

    def pair_iter(pb):
        b0, b1 = 2 * pb, 2 * pb + 1
        krot, kpas = krots[pb], kpass_[pb]
        vt0, vt1 = vts[b0], vts[b1]

        # paired rot products in col-halves; engine pick via PAIR_POOL knob
        kcr = kcpool.tile([128, CL], F16, tag="kc")
        ksr = kspool.tile([128, CL], F16, tag="ks")
        sc0 = psum_sc.tile([128, 64], F32, tag="sc", name=f"sc{b0}")
        sc1 = psum_sc.tile([128, 64], F32, tag="sc", name=f"sc{b1}")
        for half in range(2):
            lo, hi = half * HALF, (half + 1) * HALF
            kc_eng = nc.gpsimd if (pb, half, 0) in PAIR_POOL else nc.vector
            ks_eng = nc.gpsimd if (pb, half, 1) in PAIR_POOL else nc.vector
            kc_eng.tensor_mul(kcr[:, lo:hi], krot[:, lo:hi], sb_cos[:, lo:hi])
            ks_eng.tensor_mul(ksr[:, lo:hi], krot[:, lo:hi], sb_sin[:, lo:hi])
            for c in range(half * 16, half * 16 + 16):
                cs = slice(c * 128, (c + 1) * 128)
                for b, sc in ((b0, sc0), (b1, sc1)):
                    nc.tensor.matmul(sc[:, 2 * c:2 * c + 2], lhsT=kcr[:, cs],
                                     rhs=um[:, 2 * b:2 * b + 2],
                                     start=(c == 0), stop=False,
                                     skip_group_check=True)
                    nc.tensor.matmul(sc[:, 2 * c:2 * c + 2], lhsT=ksr[:, cs],
                                     rhs=vm[:, 2 * b:2 * b + 2],
                                     start=False, stop=False,
                                     skip_group_check=True)
                    nc.tensor.matmul(sc[:, 2 * c:2 * c + 2], lhsT=kpas[:, cs],
                                     rhs=upm[:, 2 * b:2 * b + 2],
                                     start=False, stop=(c == 31),
                                     skip_group_check=True)

        # exp + denominators + A.V per batch of the pair
        for b, sc, vt in ((b0, sc0, vt0), (b1, sc1, vt1)):
            at = apool.tile([128, 64], F16, tag="at")
            for h in range(H_PER_CORE):
                col = h * 8 + b
                scv = _fap(sc, h, [[2, 32]])
                nc.scalar.activation(at[:, h * 32:(h + 1) * 32], scv,
                                     AF.Exp, scale=0.125,
                                     accum_out=den_part[:, col:col + 1])
                for c in range(32):
                    nc.tensor.matmul(ov_ps[:, col:col + 1],
                                     lhsT=_fap(vt, h * 2048 + c * 64, [[1, 64]]),
                                     rhs=at[:, h * 32 + c:h * 32 + c + 1],
                                     start=False, stop=(c == 31),
                                     skip_group_check=True)

    for pb in range(4):
        pair_iter(pb)

    # denominator: column-sum of per-partition exp sums + new-token init
    nc.tensor.matmul(den_ps[:], lhsT=ones_p[:], rhs=den_part[:],
                     start=False, stop=True, skip_group_check=True)

    # ---- normalize + out-projection
    ov_sb = small.tile([64, 16], F32, tag="ov_sb")
    nc.scalar.copy(ov_sb[:], ov_ps[:])
    r_row = small.tile([1, 16], F32, tag="r_row")
    nc.vector.reciprocal(r_row[:], den_ps[:])
    r_ps = psum_r.tile([64, 16], F32, tag="r")
    nc.tensor.matmul(r_ps[:], lhsT=ones_r64[:], rhs=r_row[:], start=True, stop=True)
    on = small.tile([64, 16], F16, tag="on")
    nc.vector.tensor_mul(on[:], ov_sb[:], r_ps[:])

    out_f = small.tile([8, 1024], F32, tag="out_f")
    for nchunk in range(2):
        sl = slice(nchunk * 512, (nchunk + 1) * 512)
        ps = psum_wo.tile([8, 512], F32, tag="wo", name=f"wo_ps{nchunk}")
        nc.tensor.matmul(ps[:], lhsT=on[:, 0:8], rhs=sb_wo0[:, sl], start=True, stop=False)
        nc.tensor.matmul(ps[:], lhsT=on[:, 8:16], rhs=sb_wo1[:, sl], start=False, stop=True)
        if nchunk == 0:
            nc.vector.tensor_copy(out_f[:, sl], ps[:])
        else:
            nc.scalar.copy(out_f[:, sl], ps[:])
        (nc.sync if nchunk == 0 else nc.scalar).dma_start(out_p[:, sl], out_f[:, sl])


def _host_tables():
    """cos~/sin~ in transposed layout [128 (h,d), 4096 s] plus q-side tables."""
    inv_freq = 1.0 / (THETA ** (np.arange(0, ROT, 2, dtype=np.float64) / ROT))
    invf_rep = np.repeat(inv_freq, 2)  # [32]
    pos = np.arange(CL, dtype=np.float64)
    ang = invf_rep[:, None] * pos[None, :]  # [32 rot-d, 4096 s]
    # rows (pair-slot, h, j): the same 32 rotary rows tiled 4x
    cos_t = np.tile(np.cos(ang), (4, 1)).astype(np.float16)  # [128, 4096]
    sin_t = np.tile(np.sin(ang), (4, 1)).astype(np.float16)
    fq = 4096.0 * invf_rep
    cq_row = np.concatenate([np.cos(fq), np.ones(32)])  # per head [64]
    sq_row = np.concatenate([np.sin(fq), np.zeros(32)])
    cq_t = np.tile(np.concatenate([cq_row, cq_row]), (BS, 1)).astype(np.float32)
    sq_t = np.tile(np.concatenate([sq_row, sq_row]), (BS, 1)).astype(np.float32)
    return cos_t, sin_t, cq_t, sq_t


_NC = None


def _get_nc():
    global _NC
    if _NC is None:
        _NC = build_program()
    return _NC


def kernel(q, k_cache, v_cache, WQ_w, WQ_b, WK_w, WK_b, WV_w, WV_b, WO_w, WO_b,
           _trace=False, _tmpdir=None):
    q = np.asarray(q, dtype=np.float32)
    k16 = np.asarray(k_cache, dtype=np.float32).astype(np.float16)
    v8 = np.asarray(v_cache, dtype=np.float32).astype(ml_dtypes.float8_e3m4)
    cos_t, sin_t, cq_t, sq_t = _host_tables()
    q_t = np.ascontiguousarray(q.reshape(BS, D).T.astype(np.float16))
    id8 = np.eye(8, dtype=np.float32)
    id8f = np.eye(8, dtype=np.float16)
    # pair-slot head selectors over (slot, h, d') rows
    hsel0 = np.zeros((128, 2), np.float16)
    hsel0[0:32, 0] = 1.0
    hsel0[32:64, 1] = 1.0
    hsel1 = np.zeros((128, 2), np.float16)
    hsel1[64:96, 0] = 1.0
    hsel1[96:128, 1] = 1.0

    in_maps = []
    for c in range(N_CORES):
        sl = slice(c * 128, (c + 1) * 128)
        hs = slice(c * H_PER_CORE, (c + 1) * H_PER_CORE)
        # K: [b,h,s,d] -> [pair, rot|pass, (slot, h, d'), s]
        kk = k16[:, hs].transpose(0, 1, 3, 2)  # [b, h, d, s]
        rot = kk[:, :, 0:32].reshape(BS, 64, CL)
        pas = kk[:, :, 32:64].reshape(BS, 64, CL)
        kc = np.empty((BS // 2, 2, 128, CL), np.float16)
        for pb in range(BS // 2):
            kc[pb, 0, 0:64] = rot[2 * pb]
            kc[pb, 0, 64:128] = rot[2 * pb + 1]
            kc[pb, 1, 0:64] = pas[2 * pb]
            kc[pb, 1, 64:128] = pas[2 * pb + 1]
        # V: [b,h,s,d] -> [b, p, (h c d)] with s = c*128 + p
        vc = v8[:, hs].reshape(BS, H_PER_CORE, 32, 128, HD)
        vc = vc.transpose(0, 3, 1, 2, 4).reshape(BS, 128, CL)
        in_maps.append({
            "k_c": np.ascontiguousarray(kc),
            "v_c": np.ascontiguousarray(vc),
            "q_t": q_t,
            "wqkv_t": np.ascontiguousarray(np.concatenate(
                [np.asarray(WQ_w, np.float32)[sl].T,
                 np.asarray(WK_w, np.float32)[sl].T,
                 np.asarray(WV_w, np.float32)[sl].T], axis=1).astype(np.float16)),
            "bqkv": np.ascontiguousarray(np.concatenate(
                [np.asarray(WQ_b, np.float32)[sl],
                 np.asarray(WK_b, np.float32)[sl],
                 np.asarray(WV_b, np.float32)[sl]]).reshape(1, 384).astype(np.float16)),
            "wo_t": np.ascontiguousarray(
                np.asarray(WO_w, np.float32)[:, sl].T.astype(np.float16)),
            "cos_t": cos_t, "sin_t": sin_t, "cq_t": cq_t, "sq_t": sq_t,
            "id8": id8, "id8f": id8f, "hsel0": hsel0, "hsel1": hsel1,
        })

    nc = _get_nc()
    # the axon-tunneled device occasionally returns transient garbage/errors;
    # retry a few times, validating the result before accepting it
    out = None
    for attempt in range(4):
        try:
            res = run_bass_kernel_spmd(nc, in_maps, list(range(N_CORES)),
                                       trace=_trace, tmpdir=_tmpdir)
            partials = [np.asarray(res.results[c]["out_p"], dtype=np.float64)
                        for c in range(N_CORES)]
            cand = np.sum(partials, axis=0) + np.asarray(WO_b, np.float64)
            if np.isfinite(cand).all() and np.abs(cand).max() < 1e3:
                out = cand
                if _trace:
                    kernel._last_results = res
                break
        except Exception:
            if attempt == 3:
                raise
    if out is None:
        raise RuntimeError("kernel produced non-finite output after retries")
    return out.reshape(BS, 1, D).astype(np.float32)
